# revision 16
# baseline (speedup 1.0000x reference)
"""GRU encoder with alive-sieve freeze on 8 Trainium2 cores.

Problem: utterance [M=128, N=1024] int32 tokens, emb_table [V=32000, E=512],
GRUCell with W_ih/W_hh [3E, E], biases [3E]. Rows freeze after the step where
their token == term_id. Output: final hidden state [N, E] f32.

Strategy: data-parallel over batch (128 rows/core, batch on SBUF partitions).
Per core, per time step:
  - emb_T obtained via dma_gather(transpose=True) from a bf16 copy of the
    table: out[p, c, t] = emb[tok_t, c*128+p] -> ready-to-use matmul lhsT.
  - gi = emb @ W_ih.T (+ biases via K=1 ones-row matmuls) accumulated in PSUM,
    prefetched one step ahead; gh = state @ W_hh.T accumulates into the same
    r/z PSUM banks (n-gate kept separate for r*h_n).
  - gates on ACT (sigmoid/tanh) + DVE; the alive-freeze folds into the final
    blend: state' = (f*u0)*alive + state  with u0 = sigmoid(-p_z) = 1-z,
    f = n - state, alive a per-partition scalar from a host-precomputed mask.
  - state' transposed on PE (bf16) to feed the next step's gh.
"""

import os

import numpy as np
import ml_dtypes

M, N, V, E = 128, 1024, 32000, 512
NCORES = 8
BS = N // NCORES          # batch rows per core
KCH = E // 128            # k-chunks of the contraction dim
GB = 4                    # time steps per gather block (512 idxs/gather: 1024 hits a SWDGE descriptor limit on HW)
E3 = 3 * E

TRACE = os.environ.get("GRU_TRACE", "0") == "1"
GP_TAIL = os.environ.get("GRU_GP_TAIL", "0") == "1"
ABLATE = os.environ.get("GRU_ABLATE", "")  # "", "pe", "chain"
IMPL = os.environ.get("GRU_IMPL", "v2")  # "v1" (PE transposes) | "v2" (xbar)
WIDE_MM = os.environ.get("GRU_WIDE_MM", "0") == "1"  # 1024-free matmuls:
# illegal on HW (s3d3_mm_num_elements: matmul out must fit one PSUM bank)
XBAR = os.environ.get("GRU_XBAR", "1") == "1"  # v2: DMA transpose vs PE
LAST_RESULT = {}

_nc_cache = {}


def _build(n_steps, repeat=1):
    """repeat>1 wraps the whole GRU in an on-device For_i loop: a
    timing-only build that amortizes host/RPC overhead over `repeat`
    back-to-back executions of the full kernel body."""
    import contextlib

    import concourse.bacc as bacc
    import concourse.mybir as mybir
    import concourse.tile as tile
    from concourse.masks import make_identity

    dt = mybir.dt
    f32, bf16, i16 = dt.float32, dt.bfloat16, dt.int16
    AF = mybir.ActivationFunctionType
    OP = mybir.AluOpType

    nblk = (n_steps + GB - 1) // GB

    nc = bacc.Bacc("TRN2", target_bir_lowering=False, debug=False)

    emb = nc.dram_tensor("emb", [V, E], bf16, kind="ExternalInput")
    idx = nc.dram_tensor("idx", [128, M * BS // 16], i16, kind="ExternalInput")
    alive = nc.dram_tensor("alive", [BS, M], f32, kind="ExternalInput")
    wih = nc.dram_tensor("wih", [128, KCH, E3], bf16, kind="ExternalInput")
    whh = nc.dram_tensor("whh", [128, KCH, E3], bf16, kind="ExternalInput")
    brz = nc.dram_tensor("brz", [1, 2 * E], bf16, kind="ExternalInput")
    bin_ = nc.dram_tensor("bin", [1, E], bf16, kind="ExternalInput")
    bhn = nc.dram_tensor("bhn", [1, E], bf16, kind="ExternalInput")
    bnh = nc.dram_tensor("bnh", [1, 2 * E], bf16, kind="ExternalInput")
    out = nc.dram_tensor("out", [BS, E], f32, kind="ExternalOutput")

    with tile.TileContext(nc) as tc:
        with (
            tc.tile_pool(name="const", bufs=1) as cp,
            tc.tile_pool(name="gath", bufs=3) as gp,
            tc.tile_pool(name="work", bufs=2) as wp,
            tc.tile_pool(name="ps", bufs=2, space="PSUM") as ps,
        ):
            # ---- resident constants (idx first: gathers depend on it) ----
            idx_sb = cp.tile([128, M * BS // 16], i16)
            nc.sync.dma_start(idx_sb[:], idx[:])
            wih_sb = cp.tile([128, KCH, E3], bf16)
            nc.sync.dma_start(wih_sb[:], wih[:])
            brz_sb = cp.tile([1, 2 * E], bf16)
            nc.sync.dma_start(brz_sb[:], brz[:])
            if IMPL == "v1" or not WIDE_MM:
                bin_sb = cp.tile([1, E], bf16)
                nc.sync.dma_start(bin_sb[:], bin_[:])
                bhn_sb = cp.tile([1, E], bf16)
                nc.sync.dma_start(bhn_sb[:], bhn[:])
            else:
                bin_sb = bhn_sb = None
            if IMPL == "v2" and WIDE_MM:
                bnh_sb = cp.tile([1, 2 * E], bf16)
                nc.sync.dma_start(bnh_sb[:], bnh[:])
            else:
                bnh_sb = None
            whh_sb = cp.tile([128, KCH, E3], bf16)
            nc.scalar.dma_start(whh_sb[:], whh[:])
            alive_sb = cp.tile([BS, M], f32)
            nc.scalar.dma_start(alive_sb[:], alive[:])
            ones_sb = cp.tile([1, 128], bf16)
            nc.vector.memset(ones_sb[:], 1.0)
            if IMPL == "v1" or not XBAR:
                ident = cp.tile([128, 128], bf16)
                make_identity(nc, ident[:])
            else:
                ident = None

            rep_cm = tc.For_i(0, repeat, 1) if repeat > 1 \
                else contextlib.nullcontext()
            body_fn = _body if IMPL == "v1" else _body_v2
            with rep_cm:
                body_fn(nc, tc, cp, gp, wp, ps, n_steps, locals())

    nc.compile()
    return nc


def _body(nc, tc, cp, gp, wp, ps, n_steps, env):
    import concourse.mybir as mybir
    dt = mybir.dt
    f32, bf16 = dt.float32, dt.bfloat16
    AF = mybir.ActivationFunctionType
    OP = mybir.AluOpType
    wih_sb, whh_sb = env["wih_sb"], env["whh_sb"]
    brz_sb, bin_sb, bhn_sb = env["brz_sb"], env["bin_sb"], env["bhn_sb"]
    alive_sb, idx_sb = env["alive_sb"], env["idx_sb"]
    ones_sb, ident = env["ones_sb"], env["ident"]
    emb, out = env["emb"], env["out"]
    nblk = (n_steps + GB - 1) // GB
    if True:
        if True:
            # ---- initial state (zeros) ----
            state = wp.tile([BS, E], f32, tag="state")
            nc.vector.memset(state[:], 0.0)
            stT = wp.tile([128, E], bf16, tag="stT")
            nc.vector.memset(stT[:], 0.0)

            # ---- gather blocks (prefetched) ----
            emb_blocks = [None] * nblk

            def issue_gather(g):
                et = gp.tile([128, KCH, GB * BS], bf16, tag="embT",
                             name=f"embT_{g}")
                cols = GB * BS // 16
                nc.gpsimd.dma_gather(
                    et[:], emb[:], idx_sb[:, g * cols:(g + 1) * cols],
                    num_idxs=GB * BS, num_idxs_reg=GB * BS, elem_size=E,
                    transpose=True,
                )
                emb_blocks[g] = et

            def gi_phase(t):
                """Emit bias + input-side matmuls for step t into fresh PSUM
                tiles. Returns (Pr, Pz, Pin, Phn)."""
                Pr = ps.tile([BS, E], f32, tag="pr", name=f"pr_{t}")
                Pz = ps.tile([BS, E], f32, tag="pz", name=f"pz_{t}")
                Pin = ps.tile([BS, E], f32, tag="pin", name=f"pin_{t}")
                Phn = ps.tile([BS, E], f32, tag="phnT", name=f"phn_{t}")
                nc.tensor.matmul(Pr[:], ones_sb[:], brz_sb[:, 0:E],
                                 start=True, stop=False)
                nc.tensor.matmul(Pz[:], ones_sb[:], brz_sb[:, E:2 * E],
                                 start=True, stop=False)
                nc.tensor.matmul(Pin[:], ones_sb[:], bin_sb[:],
                                 start=True, stop=False)
                nc.tensor.matmul(Phn[:], ones_sb[:], bhn_sb[:],
                                 start=True, stop=False)
                if ABLATE != "chain":
                    et = emb_blocks[t // GB]
                    s = (t % GB) * BS
                    for k in range(KCH):
                        lhs = et[:, k, s:s + BS]
                        nc.tensor.matmul(Pr[:], lhs, wih_sb[:, k, 0:E],
                                         start=False, stop=False)
                        nc.tensor.matmul(Pz[:], lhs, wih_sb[:, k, E:2 * E],
                                         start=False, stop=False)
                        nc.tensor.matmul(Pin[:], lhs, wih_sb[:, k, 2 * E:E3],
                                         start=False, stop=(k == KCH - 1))
                return Pr, Pz, Pin, Phn

            issue_gather(0)
            if nblk > 1:
                issue_gather(1)
            cur = gi_phase(0)

            EH = E // 2          # half of the hidden dim
            H0 = slice(0, EH)
            H1 = slice(EH, E)

            for t in range(n_steps):
                Pr, Pz, Pin, Phn = cur

                # ---- gh: recurrent matmuls. r first (feeds sigmoid), then
                # hn in halves (lets d/e/tanh start on half 0 early), z last.
                for k in range(KCH if ABLATE != "chain" else 0):
                    nc.tensor.matmul(Pr[:], stT[:, k * 128:(k + 1) * 128],
                                     whh_sb[:, k, 0:E],
                                     start=False, stop=(k == KCH - 1))
                for h in ((H0, H1) if ABLATE != "chain" else ()):
                    for k in range(KCH):
                        nc.tensor.matmul(
                            Phn[:, h], stT[:, k * 128:(k + 1) * 128],
                            whh_sb[:, k, 2 * E + h.start:2 * E + h.stop],
                            start=False, stop=(k == KCH - 1 and h is H1),
                        )
                for k in range(KCH if ABLATE != "chain" else 0):
                    nc.tensor.matmul(Pz[:], stT[:, k * 128:(k + 1) * 128],
                                     whh_sb[:, k, E:2 * E],
                                     start=False, stop=(k == KCH - 1))

                # ---- prefetch: gather two blocks ahead, gi one step ahead
                if t % GB == 0 and t // GB + 2 < nblk:
                    issue_gather(t // GB + 2)
                if t + 1 < n_steps:
                    cur = gi_phase(t + 1)

                if ABLATE == "pe":
                    continue
                # ---- gates; h0 of the f/q/s' tail on DVE, h1 on GPSIMD ----
                r_sb = wp.tile([BS, E], f32, tag="r_sb", name=f"r_{t}")
                u0_sb = wp.tile([BS, E], f32, tag="u0_sb", name=f"u0_{t}")
                d_sb = wp.tile([BS, E], f32, tag="d_sb", name=f"d_{t}")
                e_sb = wp.tile([BS, E], f32, tag="e_sb", name=f"e_{t}")
                n_sb = wp.tile([BS, E], f32, tag="n_sb", name=f"n_{t}")
                f_sb = wp.tile([BS, E], f32, tag="f_sb", name=f"f_{t}")
                q_sb = wp.tile([BS, E], f32, tag="q_sb", name=f"q_{t}")
                state_new = wp.tile([BS, E], f32, tag="state", name=f"st_{t}")
                a_col = alive_sb[:, t:t + 1]

                # ACT stream: sr0, sr1, tanh0, sz0, tanh1, sz1
                nc.scalar.activation(r_sb[:, H0], Pr[:, H0], AF.Sigmoid)
                nc.scalar.activation(r_sb[:, H1], Pr[:, H1], AF.Sigmoid)
                # DVE stream: d0 e0 d1 e1 f0 q0 s0 ...
                nc.vector.tensor_tensor(d_sb[:, H0], r_sb[:, H0], Phn[:, H0],
                                        op=OP.mult)
                nc.vector.tensor_tensor(e_sb[:, H0], d_sb[:, H0], Pin[:, H0],
                                        op=OP.add)
                nc.scalar.activation(n_sb[:, H0], e_sb[:, H0], AF.Tanh)
                nc.scalar.activation(u0_sb[:, H0], Pz[:, H0], AF.Sigmoid,
                                     scale=-1.0)
                nc.vector.tensor_tensor(d_sb[:, H1], r_sb[:, H1], Phn[:, H1],
                                        op=OP.mult)
                nc.vector.tensor_tensor(e_sb[:, H1], d_sb[:, H1], Pin[:, H1],
                                        op=OP.add)
                nc.scalar.activation(n_sb[:, H1], e_sb[:, H1], AF.Tanh)
                nc.scalar.activation(u0_sb[:, H1], Pz[:, H1], AF.Sigmoid,
                                     scale=-1.0)
                # tail half 0 on DVE
                nc.vector.tensor_tensor(f_sb[:, H0], n_sb[:, H0],
                                        state[:, H0], op=OP.subtract)
                nc.vector.tensor_tensor(q_sb[:, H0], f_sb[:, H0],
                                        u0_sb[:, H0], op=OP.mult)
                nc.vector.scalar_tensor_tensor(
                    state_new[:, H0], q_sb[:, H0], a_col, state[:, H0],
                    op0=OP.mult, op1=OP.add)
                # tail half 1 (GP_TAIL picks GPSIMD vs DVE; blend on DVE:
                # TensorScalarPtr is not a Pool-engine opcode)
                eng1 = nc.gpsimd if GP_TAIL else nc.vector
                eng1.tensor_tensor(f_sb[:, H1], n_sb[:, H1],
                                   state[:, H1], op=OP.subtract)
                eng1.tensor_tensor(q_sb[:, H1], f_sb[:, H1],
                                   u0_sb[:, H1], op=OP.mult)
                nc.vector.scalar_tensor_tensor(
                    state_new[:, H1], q_sb[:, H1], a_col, state[:, H1],
                    op0=OP.mult, op1=OP.add)
                state = state_new

                # ---- transpose state for next step's gh ----
                if t + 1 < n_steps:
                    st_bf = wp.tile([BS, E], bf16, tag="st_bf", name=f"sb_{t}")
                    nc.vector.tensor_copy(st_bf[:, H0], state[:, H0])
                    eng1.tensor_copy(st_bf[:, H1], state[:, H1])
                    stT_ps = ps.tile([128, E], bf16, tag="phnT",
                                     name=f"stTp_{t}")
                    for c in range(KCH):
                        nc.tensor.transpose(
                            stT_ps[:, c * 128:(c + 1) * 128],
                            st_bf[:, c * 128:(c + 1) * 128], ident[:],
                        )
                    stT_new = wp.tile([128, E], bf16, tag="stT",
                                      name=f"stT_{t}")
                    nc.vector.tensor_copy(stT_new[:, H0], stT_ps[:, H0])
                    nc.vector.tensor_copy(stT_new[:, H1], stT_ps[:, H1])
                    stT = stT_new

            nc.sync.dma_start(out[:], state[:])


def _body_v2(nc, tc, cp, gp, wp, ps, n_steps, env):
    """v2: merged 2-bank PSUM tiles (Prz = [r|z], Pnh = [in|hn]) written in
    same-tile instruction runs; biases via two wide K=1 matmuls; f32->bf16
    casts on ACT; state transpose via XBAR DMA (sync+scalar HWDGE rings)
    instead of PE transposes + DVE copies."""
    import concourse.mybir as mybir
    dt = mybir.dt
    f32, bf16 = dt.float32, dt.bfloat16
    AF = mybir.ActivationFunctionType
    OP = mybir.AluOpType
    wih_sb, whh_sb = env["wih_sb"], env["whh_sb"]
    brz_sb, bin_sb, bhn_sb = env["brz_sb"], env["bin_sb"], env["bhn_sb"]
    bnh_sb = env["bnh_sb"]
    alive_sb, idx_sb = env["alive_sb"], env["idx_sb"]
    ones_sb = env["ones_sb"]
    emb, out = env["emb"], env["out"]
    nblk = (n_steps + GB - 1) // GB
    E2 = 2 * E
    EH = E // 2
    H0 = slice(0, EH)
    H1 = slice(EH, E)

    # ---- initial state (zeros) ----
    state = wp.tile([BS, E], f32, tag="state")
    nc.vector.memset(state[:], 0.0)
    stT = wp.tile([128, KCH, 128], bf16, tag="stT")
    nc.vector.memset(stT[:], 0.0)

    # ---- gather blocks (prefetched) ----
    emb_blocks = [None] * nblk

    def issue_gather(g):
        et = gp.tile([128, KCH, GB * BS], bf16, tag="embT", name=f"embT_{g}")
        cols = GB * BS // 16
        nc.gpsimd.dma_gather(
            et[:], emb[:], idx_sb[:, g * cols:(g + 1) * cols],
            num_idxs=GB * BS, num_idxs_reg=GB * BS, elem_size=E,
            transpose=True,
        )
        emb_blocks[g] = et

    def gi_phase(t):
        """Biases + input-side matmuls for step t into fresh paired PSUM
        tiles. Returns (Prz, Pnh)."""
        Prz = ps.tile([BS, E2], f32, tag="prz", name=f"prz_{t}")
        Pnh = ps.tile([BS, E2], f32, tag="pnh", name=f"pnh_{t}")
        if WIDE_MM:
            nc.tensor.matmul(Prz[:], ones_sb[:], brz_sb[:],
                             start=True, stop=False)
            nc.tensor.matmul(Pnh[:], ones_sb[:], bnh_sb[:],
                             start=True, stop=False)
        else:
            nc.tensor.matmul(Prz[:, 0:E], ones_sb[:], brz_sb[:, 0:E],
                             start=True, stop=False)
            nc.tensor.matmul(Prz[:, E:E2], ones_sb[:], brz_sb[:, E:E2],
                             start=True, stop=False)
            nc.tensor.matmul(Pnh[:, 0:E], ones_sb[:], bin_sb[:],
                             start=True, stop=False)
            nc.tensor.matmul(Pnh[:, E:E2], ones_sb[:], bhn_sb[:],
                             start=True, stop=False)
        if ABLATE != "chain":
            et = emb_blocks[t // GB]
            s = (t % GB) * BS
            for k in range(KCH):
                lhs = et[:, k, s:s + BS]
                if WIDE_MM:
                    nc.tensor.matmul(Prz[:], lhs, wih_sb[:, k, 0:E2],
                                     start=False, stop=False)
                else:
                    nc.tensor.matmul(Prz[:, 0:E], lhs, wih_sb[:, k, 0:E],
                                     start=False, stop=False)
                    nc.tensor.matmul(Prz[:, E:E2], lhs, wih_sb[:, k, E:E2],
                                     start=False, stop=False)
            for k in range(KCH):
                lhs = et[:, k, s:s + BS]
                nc.tensor.matmul(Pnh[:, 0:E], lhs, wih_sb[:, k, E2:E3],
                                 start=False, stop=(k == KCH - 1))
        return Prz, Pnh

    issue_gather(0)
    if nblk > 1:
        issue_gather(1)
    cur = gi_phase(0)

    for t in range(n_steps):
        Prz, Pnh = cur

        # ---- gh runs: r first (feeds sigmoid early), hn halves, z last ----
        if ABLATE != "chain":
            for k in range(KCH):
                nc.tensor.matmul(Prz[:, 0:E], stT[:, k, :],
                                 whh_sb[:, k, 0:E],
                                 start=False, stop=(k == KCH - 1))
            for h in (H0, H1):
                for k in range(KCH):
                    nc.tensor.matmul(
                        Pnh[:, E + h.start:E + h.stop], stT[:, k, :],
                        whh_sb[:, k, E2 + h.start:E2 + h.stop],
                        start=False, stop=(k == KCH - 1 and h is H1),
                    )
            for k in range(KCH):
                nc.tensor.matmul(Prz[:, E:E2], stT[:, k, :],
                                 whh_sb[:, k, E:E2],
                                 start=False, stop=(k == KCH - 1))

        # ---- prefetch: gather two blocks ahead, gi one step ahead ----
        if t % GB == 0 and t // GB + 2 < nblk:
            issue_gather(t // GB + 2)
        if t + 1 < n_steps:
            cur = gi_phase(t + 1)

        if ABLATE == "pe":
            continue
        # ---- gates ----
        r_sb = wp.tile([BS, E], f32, tag="r_sb", name=f"r_{t}")
        u0_sb = wp.tile([BS, E], f32, tag="u0_sb", name=f"u0_{t}")
        d_sb = wp.tile([BS, E], f32, tag="d_sb", name=f"d_{t}")
        e_sb = wp.tile([BS, E], f32, tag="e_sb", name=f"e_{t}")
        n_sb = wp.tile([BS, E], f32, tag="n_sb", name=f"n_{t}")
        f_sb = wp.tile([BS, E], f32, tag="f_sb", name=f"f_{t}")
        q_sb = wp.tile([BS, E], f32, tag="q_sb", name=f"q_{t}")
        state_new = wp.tile([BS, E], f32, tag="state", name=f"st_{t}")
        a_col = alive_sb[:, t:t + 1]

        Pr0, Pr1 = Prz[:, 0:EH], Prz[:, EH:E]
        Pz0, Pz1 = Prz[:, E:E + EH], Prz[:, E + EH:E2]
        Pi0, Pi1 = Pnh[:, 0:EH], Pnh[:, EH:E]
        Ph0, Ph1 = Pnh[:, E:E + EH], Pnh[:, E + EH:E2]

        # ACT stream: sr0, sr1, tanh0, sz0, tanh1, sz1, cast0, cast1
        nc.scalar.activation(r_sb[:, H0], Pr0, AF.Sigmoid)
        nc.scalar.activation(r_sb[:, H1], Pr1, AF.Sigmoid)
        # DVE stream: d0 e0 d1 e1 f0 q0 s0 f1 q1 s1
        nc.vector.tensor_tensor(d_sb[:, H0], r_sb[:, H0], Ph0, op=OP.mult)
        nc.vector.tensor_tensor(e_sb[:, H0], d_sb[:, H0], Pi0, op=OP.add)
        nc.scalar.activation(n_sb[:, H0], e_sb[:, H0], AF.Tanh)
        nc.scalar.activation(u0_sb[:, H0], Pz0, AF.Sigmoid, scale=-1.0)
        nc.vector.tensor_tensor(d_sb[:, H1], r_sb[:, H1], Ph1, op=OP.mult)
        nc.vector.tensor_tensor(e_sb[:, H1], d_sb[:, H1], Pi1, op=OP.add)
        nc.scalar.activation(n_sb[:, H1], e_sb[:, H1], AF.Tanh)
        nc.scalar.activation(u0_sb[:, H1], Pz1, AF.Sigmoid, scale=-1.0)
        nc.vector.tensor_tensor(f_sb[:, H0], n_sb[:, H0], state[:, H0],
                                op=OP.subtract)
        nc.vector.tensor_tensor(q_sb[:, H0], f_sb[:, H0], u0_sb[:, H0],
                                op=OP.mult)
        nc.vector.scalar_tensor_tensor(
            state_new[:, H0], q_sb[:, H0], a_col, state[:, H0],
            op0=OP.mult, op1=OP.add)
        nc.vector.tensor_tensor(f_sb[:, H1], n_sb[:, H1], state[:, H1],
                                op=OP.subtract)
        nc.vector.tensor_tensor(q_sb[:, H1], f_sb[:, H1], u0_sb[:, H1],
                                op=OP.mult)
        nc.vector.scalar_tensor_tensor(
            state_new[:, H1], q_sb[:, H1], a_col, state[:, H1],
            op0=OP.mult, op1=OP.add)
        state = state_new

        # ---- transpose state for next step's gh: cast on ACT, then XBAR
        # DMA transpose per half (independent e-ranges) on the two HWDGE
        # rings ----
        if t + 1 < n_steps:
            st_bf = wp.tile([BS, E], bf16, tag="st_bf", name=f"sb_{t}")
            nc.scalar.activation(st_bf[:, H0], state[:, H0], AF.Copy)
            nc.scalar.activation(st_bf[:, H1], state[:, H1], AF.Copy)
            stT_new = wp.tile([128, KCH, 128], bf16, tag="stT",
                              name=f"stT_{t}")
            if XBAR:
                nc.sync.dma_start(stT_new[:, 0:KCH // 2, :], st_bf[:, H0],
                                  transpose=True)
                nc.scalar.dma_start(stT_new[:, KCH // 2:KCH, :],
                                    st_bf[:, H1], transpose=True)
            else:
                ident = env["ident"]
                stT_ps = ps.tile([128, E], bf16, tag="pnh",
                                 name=f"stTp_{t}")
                for c in range(KCH):
                    nc.tensor.transpose(
                        stT_ps[:, c * 128:(c + 1) * 128],
                        st_bf[:, c * 128:(c + 1) * 128], ident[:],
                    )
                nc.scalar.activation(stT_new[:, 0:KCH // 2, :],
                                     stT_ps[:, 0:E // 2], AF.Copy)
                nc.scalar.activation(stT_new[:, KCH // 2:KCH, :],
                                     stT_ps[:, E // 2:E], AF.Copy)
            stT = stT_new

    nc.sync.dma_start(out[:], state[:])


def _get_nc(n_steps, repeat=1):
    key = (n_steps, repeat, IMPL, WIDE_MM, XBAR)
    if key not in _nc_cache:
        _nc_cache[key] = _build(n_steps, repeat)
    return _nc_cache[key]


def _prep_inputs(utterance, emb_table, W_ih, W_hh, b_ih, b_hh, term_id):
    """Host-side sharding/layout prep. Returns per-core in_maps."""
    utterance = np.asarray(utterance, dtype=np.int32)
    emb_table = np.asarray(emb_table, dtype=np.float32)
    W_ih = np.asarray(W_ih, dtype=np.float32)
    W_hh = np.asarray(W_hh, dtype=np.float32)
    b_ih = np.asarray(b_ih, dtype=np.float32)
    b_hh = np.asarray(b_hh, dtype=np.float32)
    term = int(np.asarray(term_id))

    bf = ml_dtypes.bfloat16
    emb_bf = np.ascontiguousarray(emb_table.astype(bf))

    def wprep(W):  # [3E, E] -> [128, KCH, 3E] with w[p,k,n] = W[n, k*128+p]
        Wt = W.T.reshape(KCH, 128, E3).transpose(1, 0, 2)
        return np.ascontiguousarray(Wt.astype(bf))

    wih_h = wprep(W_ih)
    whh_h = wprep(W_hh)
    brz_h = np.ascontiguousarray(
        (b_ih[:2 * E] + b_hh[:2 * E]).reshape(1, 2 * E).astype(bf))
    bin_h = np.ascontiguousarray(b_ih[2 * E:].reshape(1, E).astype(bf))
    bhn_h = np.ascontiguousarray(b_hh[2 * E:].reshape(1, E).astype(bf))
    bnh_h = np.ascontiguousarray(
        np.concatenate([b_ih[2 * E:], b_hh[2 * E:]]).reshape(1, 2 * E)
        .astype(bf))

    in_maps = []
    for c in range(NCORES):
        U = utterance[:, c * BS:(c + 1) * BS]          # [M, BS], (t, b)
        flat = U.reshape(-1).astype(np.int16)           # i = t*BS + b
        idx_h = np.ascontiguousarray(np.tile(flat.reshape(-1, 16).T, (8, 1)))  # [128, M*BS/16]
        hit = (U == term)
        csum = np.cumsum(hit, axis=0)
        aliveT = np.ones((M, BS), dtype=np.float32)
        aliveT[1:] = (csum[:-1] == 0)
        alive_h = np.ascontiguousarray(aliveT.T)        # [BS, M]
        in_maps.append({
            "emb": emb_bf, "idx": idx_h, "alive": alive_h,
            "wih": wih_h, "whh": whh_h,
            "brz": brz_h, "bin": bin_h, "bhn": bhn_h, "bnh": bnh_h,
        })
    return in_maps


def kernel(utterance, emb_table, W_ih, W_hh, b_ih, b_hh, term_id,
           n_steps=M):
    from concourse.bass_utils import run_bass_kernel_spmd

    nc = _get_nc(n_steps)
    in_maps = _prep_inputs(utterance, emb_table, W_ih, W_hh, b_ih, b_hh,
                           term_id)
    res = run_bass_kernel_spmd(nc, in_maps, core_ids=list(range(NCORES)),
                               trace=TRACE)
    LAST_RESULT["exec_time_ns"] = res.exec_time_ns
    LAST_RESULT["trace"] = res.instructions_and_trace
    return np.concatenate([r["out"] for r in res.results], axis=0)



# revision 21
# speedup vs baseline: 1.1476x; 1.1476x over previous
"""GRU encoder with alive-sieve freeze on 8 Trainium2 cores.

Problem: utterance [M=128, N=1024] int32 tokens, emb_table [V=32000, E=512],
GRUCell with W_ih/W_hh [3E, E], biases [3E]. Rows freeze after the step where
their token == term_id. Output: final hidden state [N, E] f32.

Strategy: data-parallel over batch (128 rows/core, batch on SBUF partitions).
Per core, per time step:
  - emb_T obtained via dma_gather(transpose=True) from a bf16 copy of the
    table: out[p, c, t] = emb[tok_t, c*128+p] -> ready-to-use matmul lhsT.
  - gi = emb @ W_ih.T (+ biases via K=1 ones-row matmuls) accumulated in PSUM,
    prefetched one step ahead; gh = state @ W_hh.T accumulates into the same
    r/z PSUM banks (n-gate kept separate for r*h_n).
  - gates on ACT (sigmoid/tanh) + DVE; the alive-freeze folds into the final
    blend: state' = (f*u0)*alive + state  with u0 = sigmoid(-p_z) = 1-z,
    f = n - state, alive a per-partition scalar from a host-precomputed mask.
  - state' transposed on PE (bf16) to feed the next step's gh.
"""

import os

import numpy as np
import ml_dtypes

M, N, V, E = 128, 1024, 32000, 512
NCORES = 8
BS = N // NCORES          # batch rows per core
KCH = E // 128            # k-chunks of the contraction dim
GB = 4                    # time steps per gather block (512 idxs/gather: 1024 hits a SWDGE descriptor limit on HW)
E3 = 3 * E

TRACE = os.environ.get("GRU_TRACE", "0") == "1"
GP_TAIL = os.environ.get("GRU_GP_TAIL", "0") == "1"
ABLATE = os.environ.get("GRU_ABLATE", "")  # "", "pe", "chain"
IMPL = os.environ.get("GRU_IMPL", "v2")  # "v1" (PE transposes) | "v2" (xbar)
WIDE_MM = os.environ.get("GRU_WIDE_MM", "0") == "1"  # 1024-free matmuls:
# illegal on HW (s3d3_mm_num_elements: matmul out must fit one PSUM bank)
XBAR = os.environ.get("GRU_XBAR", "1") == "1"  # v2: DMA transpose vs PE
RHALF = os.environ.get("GRU_RHALF", "0") == "1"  # v2: gh r-gate in halves
LAST_RESULT = {}

_nc_cache = {}


def _build(n_steps, repeat=1):
    """repeat>1 wraps the whole GRU in an on-device For_i loop: a
    timing-only build that amortizes host/RPC overhead over `repeat`
    back-to-back executions of the full kernel body."""
    import contextlib

    import concourse.bacc as bacc
    import concourse.mybir as mybir
    import concourse.tile as tile
    from concourse.masks import make_identity

    dt = mybir.dt
    f32, bf16, i16 = dt.float32, dt.bfloat16, dt.int16
    AF = mybir.ActivationFunctionType
    OP = mybir.AluOpType

    nblk = (n_steps + GB - 1) // GB

    nc = bacc.Bacc("TRN2", target_bir_lowering=False, debug=False)

    emb = nc.dram_tensor("emb", [V, E], bf16, kind="ExternalInput")
    idx = nc.dram_tensor("idx", [128, M * BS // 16], i16, kind="ExternalInput")
    alive = nc.dram_tensor("alive", [BS, M], f32, kind="ExternalInput")
    wih = nc.dram_tensor("wih", [128, KCH, E3], bf16, kind="ExternalInput")
    whh = nc.dram_tensor("whh", [128, KCH, E3], bf16, kind="ExternalInput")
    brz = nc.dram_tensor("brz", [1, 2 * E], bf16, kind="ExternalInput")
    bin_ = nc.dram_tensor("bin", [1, E], bf16, kind="ExternalInput")
    bhn = nc.dram_tensor("bhn", [1, E], bf16, kind="ExternalInput")
    bnh = nc.dram_tensor("bnh", [1, 2 * E], bf16, kind="ExternalInput")
    out = nc.dram_tensor("out", [BS, E], f32, kind="ExternalOutput")

    with tile.TileContext(nc) as tc:
        with (
            tc.tile_pool(name="const", bufs=1) as cp,
            tc.tile_pool(name="gath", bufs=3) as gp,
            tc.tile_pool(name="work", bufs=2) as wp,
            tc.tile_pool(name="ps", bufs=2, space="PSUM") as ps,
        ):
            # ---- resident constants (idx first: gathers depend on it) ----
            idx_sb = cp.tile([128, M * BS // 16], i16)
            nc.sync.dma_start(idx_sb[:], idx[:])
            wih_sb = cp.tile([128, KCH, E3], bf16)
            nc.sync.dma_start(wih_sb[:], wih[:])
            brz_sb = cp.tile([1, 2 * E], bf16)
            nc.sync.dma_start(brz_sb[:], brz[:])
            if IMPL == "v1" or not WIDE_MM:
                bin_sb = cp.tile([1, E], bf16)
                nc.sync.dma_start(bin_sb[:], bin_[:])
                bhn_sb = cp.tile([1, E], bf16)
                nc.sync.dma_start(bhn_sb[:], bhn[:])
            else:
                bin_sb = bhn_sb = None
            if IMPL == "v2" and WIDE_MM:
                bnh_sb = cp.tile([1, 2 * E], bf16)
                nc.sync.dma_start(bnh_sb[:], bnh[:])
            else:
                bnh_sb = None
            whh_sb = cp.tile([128, KCH, E3], bf16)
            nc.scalar.dma_start(whh_sb[:], whh[:])
            alive_sb = cp.tile([BS, M], f32)
            nc.scalar.dma_start(alive_sb[:], alive[:])
            ones_sb = cp.tile([1, 128], bf16)
            nc.vector.memset(ones_sb[:], 1.0)
            if IMPL == "v1" or not XBAR:
                ident = cp.tile([128, 128], bf16)
                make_identity(nc, ident[:])
            else:
                ident = None

            rep_cm = tc.For_i(0, repeat, 1) if repeat > 1 \
                else contextlib.nullcontext()
            body_fn = _body if IMPL == "v1" else _body_v2
            with rep_cm:
                body_fn(nc, tc, cp, gp, wp, ps, n_steps, locals())

    nc.compile()
    return nc


def _body(nc, tc, cp, gp, wp, ps, n_steps, env):
    import concourse.mybir as mybir
    dt = mybir.dt
    f32, bf16 = dt.float32, dt.bfloat16
    AF = mybir.ActivationFunctionType
    OP = mybir.AluOpType
    wih_sb, whh_sb = env["wih_sb"], env["whh_sb"]
    brz_sb, bin_sb, bhn_sb = env["brz_sb"], env["bin_sb"], env["bhn_sb"]
    alive_sb, idx_sb = env["alive_sb"], env["idx_sb"]
    ones_sb, ident = env["ones_sb"], env["ident"]
    emb, out = env["emb"], env["out"]
    nblk = (n_steps + GB - 1) // GB
    if True:
        if True:
            # ---- initial state (zeros) ----
            state = wp.tile([BS, E], f32, tag="state")
            nc.vector.memset(state[:], 0.0)
            stT = wp.tile([128, E], bf16, tag="stT")
            nc.vector.memset(stT[:], 0.0)

            # ---- gather blocks (prefetched) ----
            emb_blocks = [None] * nblk

            def issue_gather(g):
                et = gp.tile([128, KCH, GB * BS], bf16, tag="embT",
                             name=f"embT_{g}")
                cols = GB * BS // 16
                nc.gpsimd.dma_gather(
                    et[:], emb[:], idx_sb[:, g * cols:(g + 1) * cols],
                    num_idxs=GB * BS, num_idxs_reg=GB * BS, elem_size=E,
                    transpose=True,
                )
                emb_blocks[g] = et

            def gi_phase(t):
                """Emit bias + input-side matmuls for step t into fresh PSUM
                tiles. Returns (Pr, Pz, Pin, Phn)."""
                Pr = ps.tile([BS, E], f32, tag="pr", name=f"pr_{t}")
                Pz = ps.tile([BS, E], f32, tag="pz", name=f"pz_{t}")
                Pin = ps.tile([BS, E], f32, tag="pin", name=f"pin_{t}")
                Phn = ps.tile([BS, E], f32, tag="phnT", name=f"phn_{t}")
                nc.tensor.matmul(Pr[:], ones_sb[:], brz_sb[:, 0:E],
                                 start=True, stop=False)
                nc.tensor.matmul(Pz[:], ones_sb[:], brz_sb[:, E:2 * E],
                                 start=True, stop=False)
                nc.tensor.matmul(Pin[:], ones_sb[:], bin_sb[:],
                                 start=True, stop=False)
                nc.tensor.matmul(Phn[:], ones_sb[:], bhn_sb[:],
                                 start=True, stop=False)
                if ABLATE != "chain":
                    et = emb_blocks[t // GB]
                    s = (t % GB) * BS
                    for k in range(KCH):
                        lhs = et[:, k, s:s + BS]
                        nc.tensor.matmul(Pr[:], lhs, wih_sb[:, k, 0:E],
                                         start=False, stop=False)
                        nc.tensor.matmul(Pz[:], lhs, wih_sb[:, k, E:2 * E],
                                         start=False, stop=False)
                        nc.tensor.matmul(Pin[:], lhs, wih_sb[:, k, 2 * E:E3],
                                         start=False, stop=(k == KCH - 1))
                return Pr, Pz, Pin, Phn

            issue_gather(0)
            if nblk > 1:
                issue_gather(1)
            cur = gi_phase(0)

            EH = E // 2          # half of the hidden dim
            H0 = slice(0, EH)
            H1 = slice(EH, E)

            for t in range(n_steps):
                Pr, Pz, Pin, Phn = cur

                # ---- gh: recurrent matmuls. r first (feeds sigmoid), then
                # hn in halves (lets d/e/tanh start on half 0 early), z last.
                for k in range(KCH if ABLATE != "chain" else 0):
                    nc.tensor.matmul(Pr[:], stT[:, k * 128:(k + 1) * 128],
                                     whh_sb[:, k, 0:E],
                                     start=False, stop=(k == KCH - 1))
                for h in ((H0, H1) if ABLATE != "chain" else ()):
                    for k in range(KCH):
                        nc.tensor.matmul(
                            Phn[:, h], stT[:, k * 128:(k + 1) * 128],
                            whh_sb[:, k, 2 * E + h.start:2 * E + h.stop],
                            start=False, stop=(k == KCH - 1 and h is H1),
                        )
                for k in range(KCH if ABLATE != "chain" else 0):
                    nc.tensor.matmul(Pz[:], stT[:, k * 128:(k + 1) * 128],
                                     whh_sb[:, k, E:2 * E],
                                     start=False, stop=(k == KCH - 1))

                # ---- prefetch: gather two blocks ahead, gi one step ahead
                if t % GB == 0 and t // GB + 2 < nblk:
                    issue_gather(t // GB + 2)
                if t + 1 < n_steps:
                    cur = gi_phase(t + 1)

                if ABLATE == "pe":
                    continue
                # ---- gates; h0 of the f/q/s' tail on DVE, h1 on GPSIMD ----
                r_sb = wp.tile([BS, E], f32, tag="r_sb", name=f"r_{t}")
                u0_sb = wp.tile([BS, E], f32, tag="u0_sb", name=f"u0_{t}")
                d_sb = wp.tile([BS, E], f32, tag="d_sb", name=f"d_{t}")
                e_sb = wp.tile([BS, E], f32, tag="e_sb", name=f"e_{t}")
                n_sb = wp.tile([BS, E], f32, tag="n_sb", name=f"n_{t}")
                f_sb = wp.tile([BS, E], f32, tag="f_sb", name=f"f_{t}")
                q_sb = wp.tile([BS, E], f32, tag="q_sb", name=f"q_{t}")
                state_new = wp.tile([BS, E], f32, tag="state", name=f"st_{t}")
                a_col = alive_sb[:, t:t + 1]

                # ACT stream: sr0, sr1, tanh0, sz0, tanh1, sz1
                nc.scalar.activation(r_sb[:, H0], Pr[:, H0], AF.Sigmoid)
                nc.scalar.activation(r_sb[:, H1], Pr[:, H1], AF.Sigmoid)
                # DVE stream: d0 e0 d1 e1 f0 q0 s0 ...
                nc.vector.tensor_tensor(d_sb[:, H0], r_sb[:, H0], Phn[:, H0],
                                        op=OP.mult)
                nc.vector.tensor_tensor(e_sb[:, H0], d_sb[:, H0], Pin[:, H0],
                                        op=OP.add)
                nc.scalar.activation(n_sb[:, H0], e_sb[:, H0], AF.Tanh)
                nc.scalar.activation(u0_sb[:, H0], Pz[:, H0], AF.Sigmoid,
                                     scale=-1.0)
                nc.vector.tensor_tensor(d_sb[:, H1], r_sb[:, H1], Phn[:, H1],
                                        op=OP.mult)
                nc.vector.tensor_tensor(e_sb[:, H1], d_sb[:, H1], Pin[:, H1],
                                        op=OP.add)
                nc.scalar.activation(n_sb[:, H1], e_sb[:, H1], AF.Tanh)
                nc.scalar.activation(u0_sb[:, H1], Pz[:, H1], AF.Sigmoid,
                                     scale=-1.0)
                # tail half 0 on DVE
                nc.vector.tensor_tensor(f_sb[:, H0], n_sb[:, H0],
                                        state[:, H0], op=OP.subtract)
                nc.vector.tensor_tensor(q_sb[:, H0], f_sb[:, H0],
                                        u0_sb[:, H0], op=OP.mult)
                nc.vector.scalar_tensor_tensor(
                    state_new[:, H0], q_sb[:, H0], a_col, state[:, H0],
                    op0=OP.mult, op1=OP.add)
                # tail half 1 (GP_TAIL picks GPSIMD vs DVE; blend on DVE:
                # TensorScalarPtr is not a Pool-engine opcode)
                eng1 = nc.gpsimd if GP_TAIL else nc.vector
                eng1.tensor_tensor(f_sb[:, H1], n_sb[:, H1],
                                   state[:, H1], op=OP.subtract)
                eng1.tensor_tensor(q_sb[:, H1], f_sb[:, H1],
                                   u0_sb[:, H1], op=OP.mult)
                nc.vector.scalar_tensor_tensor(
                    state_new[:, H1], q_sb[:, H1], a_col, state[:, H1],
                    op0=OP.mult, op1=OP.add)
                state = state_new

                # ---- transpose state for next step's gh ----
                if t + 1 < n_steps:
                    st_bf = wp.tile([BS, E], bf16, tag="st_bf", name=f"sb_{t}")
                    nc.vector.tensor_copy(st_bf[:, H0], state[:, H0])
                    eng1.tensor_copy(st_bf[:, H1], state[:, H1])
                    stT_ps = ps.tile([128, E], bf16, tag="phnT",
                                     name=f"stTp_{t}")
                    for c in range(KCH):
                        nc.tensor.transpose(
                            stT_ps[:, c * 128:(c + 1) * 128],
                            st_bf[:, c * 128:(c + 1) * 128], ident[:],
                        )
                    stT_new = wp.tile([128, E], bf16, tag="stT",
                                      name=f"stT_{t}")
                    nc.vector.tensor_copy(stT_new[:, H0], stT_ps[:, H0])
                    nc.vector.tensor_copy(stT_new[:, H1], stT_ps[:, H1])
                    stT = stT_new

            nc.sync.dma_start(out[:], state[:])


def _body_v2(nc, tc, cp, gp, wp, ps, n_steps, env):
    """v2: merged 2-bank PSUM tiles (Prz = [r|z], Pnh = [in|hn]) written in
    same-tile instruction runs; biases via two wide K=1 matmuls; f32->bf16
    casts on ACT; state transpose via XBAR DMA (sync+scalar HWDGE rings)
    instead of PE transposes + DVE copies."""
    import concourse.mybir as mybir
    dt = mybir.dt
    f32, bf16 = dt.float32, dt.bfloat16
    AF = mybir.ActivationFunctionType
    OP = mybir.AluOpType
    wih_sb, whh_sb = env["wih_sb"], env["whh_sb"]
    brz_sb, bin_sb, bhn_sb = env["brz_sb"], env["bin_sb"], env["bhn_sb"]
    bnh_sb = env["bnh_sb"]
    alive_sb, idx_sb = env["alive_sb"], env["idx_sb"]
    ones_sb = env["ones_sb"]
    emb, out = env["emb"], env["out"]
    nblk = (n_steps + GB - 1) // GB
    E2 = 2 * E
    EH = E // 2
    H0 = slice(0, EH)
    H1 = slice(EH, E)

    # ---- initial state (zeros) ----
    state = wp.tile([BS, E], f32, tag="state")
    nc.vector.memset(state[:], 0.0)
    stT = wp.tile([128, KCH, 128], bf16, tag="stT")
    nc.vector.memset(stT[:], 0.0)

    # ---- gather blocks (prefetched) ----
    emb_blocks = [None] * nblk

    def issue_gather(g):
        et = gp.tile([128, KCH, GB * BS], bf16, tag="embT", name=f"embT_{g}")
        cols = GB * BS // 16
        nc.gpsimd.dma_gather(
            et[:], emb[:], idx_sb[:, g * cols:(g + 1) * cols],
            num_idxs=GB * BS, num_idxs_reg=GB * BS, elem_size=E,
            transpose=True,
        )
        emb_blocks[g] = et

    def gi_phase(t):
        """Biases + input-side matmuls for step t into fresh paired PSUM
        tiles. Returns (Prz, Pnh)."""
        Prz = ps.tile([BS, E2], f32, tag="prz", name=f"prz_{t}")
        Pnh = ps.tile([BS, E2], f32, tag="pnh", name=f"pnh_{t}")
        if WIDE_MM:
            nc.tensor.matmul(Prz[:], ones_sb[:], brz_sb[:],
                             start=True, stop=False)
            nc.tensor.matmul(Pnh[:], ones_sb[:], bnh_sb[:],
                             start=True, stop=False)
        else:
            nc.tensor.matmul(Prz[:, 0:E], ones_sb[:], brz_sb[:, 0:E],
                             start=True, stop=False)
            nc.tensor.matmul(Prz[:, E:E2], ones_sb[:], brz_sb[:, E:E2],
                             start=True, stop=False)
            nc.tensor.matmul(Pnh[:, 0:E], ones_sb[:], bin_sb[:],
                             start=True, stop=False)
            nc.tensor.matmul(Pnh[:, E:E2], ones_sb[:], bhn_sb[:],
                             start=True, stop=False)
        if ABLATE != "chain":
            et = emb_blocks[t // GB]
            s = (t % GB) * BS
            for k in range(KCH):
                lhs = et[:, k, s:s + BS]
                if WIDE_MM:
                    nc.tensor.matmul(Prz[:], lhs, wih_sb[:, k, 0:E2],
                                     start=False, stop=False)
                else:
                    nc.tensor.matmul(Prz[:, 0:E], lhs, wih_sb[:, k, 0:E],
                                     start=False, stop=False)
                    nc.tensor.matmul(Prz[:, E:E2], lhs, wih_sb[:, k, E:E2],
                                     start=False, stop=False)
            for k in range(KCH):
                lhs = et[:, k, s:s + BS]
                nc.tensor.matmul(Pnh[:, 0:E], lhs, wih_sb[:, k, E2:E3],
                                 start=False, stop=(k == KCH - 1))
        return Prz, Pnh

    issue_gather(0)
    if nblk > 1:
        issue_gather(1)
    cur = gi_phase(0)

    for t in range(n_steps):
        Prz, Pnh = cur

        # ---- gh runs: r first (feeds sigmoid early), hn halves, z last ----
        if ABLATE != "chain":
            if RHALF:
                for h in (H0, H1):
                    for k in range(KCH):
                        nc.tensor.matmul(
                            Prz[:, h], stT[:, k, :],
                            whh_sb[:, k, h.start:h.stop],
                            start=False, stop=(k == KCH - 1 and h is H1))
            else:
                for k in range(KCH):
                    nc.tensor.matmul(Prz[:, 0:E], stT[:, k, :],
                                     whh_sb[:, k, 0:E],
                                     start=False, stop=(k == KCH - 1))
            for h in (H0, H1):
                for k in range(KCH):
                    nc.tensor.matmul(
                        Pnh[:, E + h.start:E + h.stop], stT[:, k, :],
                        whh_sb[:, k, E2 + h.start:E2 + h.stop],
                        start=False, stop=(k == KCH - 1 and h is H1),
                    )
            for k in range(KCH):
                nc.tensor.matmul(Prz[:, E:E2], stT[:, k, :],
                                 whh_sb[:, k, E:E2],
                                 start=False, stop=(k == KCH - 1))

        # ---- prefetch: gather two blocks ahead, gi one step ahead ----
        if t % GB == 0 and t // GB + 2 < nblk:
            issue_gather(t // GB + 2)
        if t + 1 < n_steps:
            cur = gi_phase(t + 1)

        if ABLATE == "pe":
            continue
        # ---- gates ----
        r_sb = wp.tile([BS, E], f32, tag="r_sb", name=f"r_{t}")
        u0_sb = wp.tile([BS, E], f32, tag="u0_sb", name=f"u0_{t}")
        d_sb = wp.tile([BS, E], f32, tag="d_sb", name=f"d_{t}")
        e_sb = wp.tile([BS, E], f32, tag="e_sb", name=f"e_{t}")
        n_sb = wp.tile([BS, E], f32, tag="n_sb", name=f"n_{t}")
        f_sb = wp.tile([BS, E], f32, tag="f_sb", name=f"f_{t}")
        q_sb = wp.tile([BS, E], f32, tag="q_sb", name=f"q_{t}")
        state_new = wp.tile([BS, E], f32, tag="state", name=f"st_{t}")
        a_col = alive_sb[:, t:t + 1]

        Pr0, Pr1 = Prz[:, 0:EH], Prz[:, EH:E]
        Pz0, Pz1 = Prz[:, E:E + EH], Prz[:, E + EH:E2]
        Pi0, Pi1 = Pnh[:, 0:EH], Pnh[:, EH:E]
        Ph0, Ph1 = Pnh[:, E:E + EH], Pnh[:, E + EH:E2]

        # ACT stream: sr0, sr1, tanh0, sz0, tanh1, sz1, cast0, cast1
        nc.scalar.activation(r_sb[:, H0], Pr0, AF.Sigmoid)
        nc.scalar.activation(r_sb[:, H1], Pr1, AF.Sigmoid)
        # DVE stream: d0 e0 d1 e1 f0 q0 s0 f1 q1 s1
        nc.vector.tensor_tensor(d_sb[:, H0], r_sb[:, H0], Ph0, op=OP.mult)
        nc.vector.tensor_tensor(e_sb[:, H0], d_sb[:, H0], Pi0, op=OP.add)
        nc.scalar.activation(n_sb[:, H0], e_sb[:, H0], AF.Tanh)
        nc.scalar.activation(u0_sb[:, H0], Pz0, AF.Sigmoid, scale=-1.0)
        nc.vector.tensor_tensor(d_sb[:, H1], r_sb[:, H1], Ph1, op=OP.mult)
        nc.vector.tensor_tensor(e_sb[:, H1], d_sb[:, H1], Pi1, op=OP.add)
        nc.scalar.activation(n_sb[:, H1], e_sb[:, H1], AF.Tanh)
        nc.scalar.activation(u0_sb[:, H1], Pz1, AF.Sigmoid, scale=-1.0)
        nc.vector.tensor_tensor(f_sb[:, H0], n_sb[:, H0], state[:, H0],
                                op=OP.subtract)
        nc.vector.tensor_tensor(q_sb[:, H0], f_sb[:, H0], u0_sb[:, H0],
                                op=OP.mult)
        nc.vector.scalar_tensor_tensor(
            state_new[:, H0], q_sb[:, H0], a_col, state[:, H0],
            op0=OP.mult, op1=OP.add)
        nc.vector.tensor_tensor(f_sb[:, H1], n_sb[:, H1], state[:, H1],
                                op=OP.subtract)
        nc.vector.tensor_tensor(q_sb[:, H1], f_sb[:, H1], u0_sb[:, H1],
                                op=OP.mult)
        nc.vector.scalar_tensor_tensor(
            state_new[:, H1], q_sb[:, H1], a_col, state[:, H1],
            op0=OP.mult, op1=OP.add)
        state = state_new

        # ---- transpose state for next step's gh: cast on ACT, then XBAR
        # DMA transpose per half (independent e-ranges) on the two HWDGE
        # rings ----
        if t + 1 < n_steps:
            st_bf = wp.tile([BS, E], bf16, tag="st_bf", name=f"sb_{t}")
            nc.vector.tensor_copy(st_bf[:, H0], state[:, H0])
            nc.vector.tensor_copy(st_bf[:, H1], state[:, H1])
            stT_new = wp.tile([128, KCH, 128], bf16, tag="stT",
                              name=f"stT_{t}")
            if XBAR:
                nc.sync.dma_start(stT_new[:], st_bf[:], transpose=True)
            else:
                ident = env["ident"]
                stT_ps = ps.tile([128, E], bf16, tag="pnh",
                                 name=f"stTp_{t}")
                for c in range(KCH):
                    nc.tensor.transpose(
                        stT_ps[:, c * 128:(c + 1) * 128],
                        st_bf[:, c * 128:(c + 1) * 128], ident[:],
                    )
                nc.vector.tensor_copy(stT_new[:, 0:KCH // 2, :],
                                      stT_ps[:, 0:E // 2])
                nc.vector.tensor_copy(stT_new[:, KCH // 2:KCH, :],
                                      stT_ps[:, E // 2:E])
            stT = stT_new

    nc.sync.dma_start(out[:], state[:])


def _get_nc(n_steps, repeat=1):
    key = (n_steps, repeat, IMPL, WIDE_MM, XBAR, RHALF)
    if key not in _nc_cache:
        _nc_cache[key] = _build(n_steps, repeat)
    return _nc_cache[key]


def _prep_inputs(utterance, emb_table, W_ih, W_hh, b_ih, b_hh, term_id):
    """Host-side sharding/layout prep. Returns per-core in_maps."""
    utterance = np.asarray(utterance, dtype=np.int32)
    emb_table = np.asarray(emb_table, dtype=np.float32)
    W_ih = np.asarray(W_ih, dtype=np.float32)
    W_hh = np.asarray(W_hh, dtype=np.float32)
    b_ih = np.asarray(b_ih, dtype=np.float32)
    b_hh = np.asarray(b_hh, dtype=np.float32)
    term = int(np.asarray(term_id))

    bf = ml_dtypes.bfloat16
    emb_bf = np.ascontiguousarray(emb_table.astype(bf))

    def wprep(W):  # [3E, E] -> [128, KCH, 3E] with w[p,k,n] = W[n, k*128+p]
        Wt = W.T.reshape(KCH, 128, E3).transpose(1, 0, 2)
        return np.ascontiguousarray(Wt.astype(bf))

    wih_h = wprep(W_ih)
    whh_h = wprep(W_hh)
    brz_h = np.ascontiguousarray(
        (b_ih[:2 * E] + b_hh[:2 * E]).reshape(1, 2 * E).astype(bf))
    bin_h = np.ascontiguousarray(b_ih[2 * E:].reshape(1, E).astype(bf))
    bhn_h = np.ascontiguousarray(b_hh[2 * E:].reshape(1, E).astype(bf))
    bnh_h = np.ascontiguousarray(
        np.concatenate([b_ih[2 * E:], b_hh[2 * E:]]).reshape(1, 2 * E)
        .astype(bf))

    in_maps = []
    for c in range(NCORES):
        U = utterance[:, c * BS:(c + 1) * BS]          # [M, BS], (t, b)
        flat = U.reshape(-1).astype(np.int16)           # i = t*BS + b
        idx_h = np.ascontiguousarray(np.tile(flat.reshape(-1, 16).T, (8, 1)))  # [128, M*BS/16]
        hit = (U == term)
        csum = np.cumsum(hit, axis=0)
        aliveT = np.ones((M, BS), dtype=np.float32)
        aliveT[1:] = (csum[:-1] == 0)
        alive_h = np.ascontiguousarray(aliveT.T)        # [BS, M]
        in_maps.append({
            "emb": emb_bf, "idx": idx_h, "alive": alive_h,
            "wih": wih_h, "whh": whh_h,
            "brz": brz_h, "bin": bin_h, "bhn": bhn_h, "bnh": bnh_h,
        })
    return in_maps


def kernel(utterance, emb_table, W_ih, W_hh, b_ih, b_hh, term_id,
           n_steps=M):
    from concourse.bass_utils import run_bass_kernel_spmd

    nc = _get_nc(n_steps)
    in_maps = _prep_inputs(utterance, emb_table, W_ih, W_hh, b_ih, b_hh,
                           term_id)
    res = run_bass_kernel_spmd(nc, in_maps, core_ids=list(range(NCORES)),
                               trace=TRACE)
    LAST_RESULT["exec_time_ns"] = res.exec_time_ns
    LAST_RESULT["trace"] = res.instructions_and_trace
    return np.concatenate([r["out"] for r in res.results], axis=0)



# revision 26
# speedup vs baseline: 1.6905x; 1.4730x over previous
"""GRU encoder with alive-sieve freeze on 8 Trainium2 cores.

Problem: utterance [M=128, N=1024] int32 tokens, emb_table [V=32000, E=512],
GRUCell with W_ih/W_hh [3E, E], biases [3E]. Rows freeze after the step where
their token == term_id. Output: final hidden state [N, E] f32.

Strategy: data-parallel over batch (128 rows/core, batch on SBUF partitions).
Per core, per time step:
  - emb_T obtained via dma_gather(transpose=True) from a bf16 copy of the
    table: out[p, c, t] = emb[tok_t, c*128+p] -> ready-to-use matmul lhsT.
  - gi = emb @ W_ih.T (+ biases via K=1 ones-row matmuls) accumulated in PSUM,
    prefetched one step ahead; gh = state @ W_hh.T accumulates into the same
    r/z PSUM banks (n-gate kept separate for r*h_n).
  - gates on ACT (sigmoid/tanh) + DVE; the alive-freeze folds into the final
    blend: state' = (f*u0)*alive + state  with u0 = sigmoid(-p_z) = 1-z,
    f = n - state, alive a per-partition scalar from a host-precomputed mask.
  - state' transposed on PE (bf16) to feed the next step's gh.
"""

import os

import numpy as np
import ml_dtypes

M, N, V, E = 128, 1024, 32000, 512
NCORES = 8
BS = N // NCORES          # batch rows per core
KCH = E // 128            # k-chunks of the contraction dim
GB = 4                    # time steps per gather block (512 idxs/gather: 1024 hits a SWDGE descriptor limit on HW)
E3 = 3 * E

TRACE = os.environ.get("GRU_TRACE", "0") == "1"
GP_TAIL = os.environ.get("GRU_GP_TAIL", "0") == "1"
ABLATE = os.environ.get("GRU_ABLATE", "")  # "", "pe", "chain"
IMPL = os.environ.get("GRU_IMPL", "v2")  # "v1" (PE transposes) | "v2" (xbar)
WIDE_MM = os.environ.get("GRU_WIDE_MM", "0") == "1"  # 1024-free matmuls:
# illegal on HW (s3d3_mm_num_elements: matmul out must fit one PSUM bank)
XBAR = os.environ.get("GRU_XBAR", "1") == "1"  # v2: DMA transpose vs PE
RHALF = os.environ.get("GRU_RHALF", "0") == "1"  # v2: gh r-gate in halves
LAST_RESULT = {}

_nc_cache = {}


def _build(n_steps, repeat=1):
    """repeat>1 wraps the whole GRU in an on-device For_i loop: a
    timing-only build that amortizes host/RPC overhead over `repeat`
    back-to-back executions of the full kernel body."""
    import contextlib

    import concourse.bacc as bacc
    import concourse.mybir as mybir
    import concourse.tile as tile
    from concourse.masks import make_identity

    dt = mybir.dt
    f32, bf16, i16 = dt.float32, dt.bfloat16, dt.int16
    AF = mybir.ActivationFunctionType
    OP = mybir.AluOpType

    nblk = (n_steps + GB - 1) // GB

    nc = bacc.Bacc("TRN2", target_bir_lowering=False, debug=False)

    emb = nc.dram_tensor("emb", [V, E], bf16, kind="ExternalInput")
    idx = nc.dram_tensor("idx", [128, M * BS // 16], i16, kind="ExternalInput")
    alive = nc.dram_tensor("alive", [BS, M], f32, kind="ExternalInput")
    wih = nc.dram_tensor("wih", [128, KCH, E3], bf16, kind="ExternalInput")
    whh = nc.dram_tensor("whh", [128, KCH, E3], bf16, kind="ExternalInput")
    brz = nc.dram_tensor("brz", [1, 2 * E], bf16, kind="ExternalInput")
    bin_ = nc.dram_tensor("bin", [1, E], bf16, kind="ExternalInput")
    bhn = nc.dram_tensor("bhn", [1, E], bf16, kind="ExternalInput")
    bnh = nc.dram_tensor("bnh", [1, 2 * E], bf16, kind="ExternalInput")
    out = nc.dram_tensor("out", [BS, E], f32, kind="ExternalOutput")

    with tile.TileContext(nc) as tc:
        with (
            tc.tile_pool(name="const", bufs=1) as cp,
            tc.tile_pool(name="gath", bufs=3) as gp,
            tc.tile_pool(name="work", bufs=2) as wp,
            tc.tile_pool(name="ps", bufs=2, space="PSUM") as ps,
        ):
            # ---- resident constants (idx first: gathers depend on it) ----
            idx_sb = cp.tile([128, M * BS // 16], i16)
            nc.sync.dma_start(idx_sb[:], idx[:])
            wih_sb = cp.tile([128, KCH, E3], bf16)
            nc.sync.dma_start(wih_sb[:], wih[:])
            brz_sb = cp.tile([1, 2 * E], bf16)
            nc.sync.dma_start(brz_sb[:], brz[:])
            if IMPL == "v1" or not WIDE_MM:
                bin_sb = cp.tile([1, E], bf16)
                nc.sync.dma_start(bin_sb[:], bin_[:])
                bhn_sb = cp.tile([1, E], bf16)
                nc.sync.dma_start(bhn_sb[:], bhn[:])
            else:
                bin_sb = bhn_sb = None
            if IMPL == "v2" and WIDE_MM:
                bnh_sb = cp.tile([1, 2 * E], bf16)
                nc.sync.dma_start(bnh_sb[:], bnh[:])
            else:
                bnh_sb = None
            whh_sb = cp.tile([128, KCH, E3], bf16)
            nc.scalar.dma_start(whh_sb[:], whh[:])
            alive_sb = cp.tile([BS, M], f32)
            nc.scalar.dma_start(alive_sb[:], alive[:])
            ones_sb = cp.tile([1, 128], bf16)
            nc.vector.memset(ones_sb[:], 1.0)
            if IMPL == "v1" or not XBAR:
                ident = cp.tile([128, 128], bf16)
                make_identity(nc, ident[:])
            else:
                ident = None

            rep_cm = tc.For_i(0, repeat, 1) if repeat > 1 \
                else contextlib.nullcontext()
            body_fn = _body if IMPL == "v1" else _body_v2
            with rep_cm:
                body_fn(nc, tc, cp, gp, wp, ps, n_steps, locals())

    nc.compile()
    return nc


def _body(nc, tc, cp, gp, wp, ps, n_steps, env):
    import concourse.mybir as mybir
    dt = mybir.dt
    f32, bf16 = dt.float32, dt.bfloat16
    AF = mybir.ActivationFunctionType
    OP = mybir.AluOpType
    wih_sb, whh_sb = env["wih_sb"], env["whh_sb"]
    brz_sb, bin_sb, bhn_sb = env["brz_sb"], env["bin_sb"], env["bhn_sb"]
    alive_sb, idx_sb = env["alive_sb"], env["idx_sb"]
    ones_sb, ident = env["ones_sb"], env["ident"]
    emb, out = env["emb"], env["out"]
    nblk = (n_steps + GB - 1) // GB
    if True:
        if True:
            # ---- initial state (zeros) ----
            state = wp.tile([BS, E], f32, tag="state")
            nc.vector.memset(state[:], 0.0)
            stT = wp.tile([128, E], bf16, tag="stT")
            nc.vector.memset(stT[:], 0.0)

            # ---- gather blocks (prefetched) ----
            emb_blocks = [None] * nblk

            def issue_gather(g):
                et = gp.tile([128, KCH, GB * BS], bf16, tag="embT",
                             name=f"embT_{g}")
                cols = GB * BS // 16
                nc.gpsimd.dma_gather(
                    et[:], emb[:], idx_sb[:, g * cols:(g + 1) * cols],
                    num_idxs=GB * BS, num_idxs_reg=GB * BS, elem_size=E,
                    transpose=True,
                )
                emb_blocks[g] = et

            def gi_phase(t):
                """Emit bias + input-side matmuls for step t into fresh PSUM
                tiles. Returns (Pr, Pz, Pin, Phn)."""
                Pr = ps.tile([BS, E], f32, tag="pr", name=f"pr_{t}")
                Pz = ps.tile([BS, E], f32, tag="pz", name=f"pz_{t}")
                Pin = ps.tile([BS, E], f32, tag="pin", name=f"pin_{t}")
                Phn = ps.tile([BS, E], f32, tag="phnT", name=f"phn_{t}")
                nc.tensor.matmul(Pr[:], ones_sb[:], brz_sb[:, 0:E],
                                 start=True, stop=False)
                nc.tensor.matmul(Pz[:], ones_sb[:], brz_sb[:, E:2 * E],
                                 start=True, stop=False)
                nc.tensor.matmul(Pin[:], ones_sb[:], bin_sb[:],
                                 start=True, stop=False)
                nc.tensor.matmul(Phn[:], ones_sb[:], bhn_sb[:],
                                 start=True, stop=False)
                if ABLATE != "chain":
                    et = emb_blocks[t // GB]
                    s = (t % GB) * BS
                    for k in range(KCH):
                        lhs = et[:, k, s:s + BS]
                        nc.tensor.matmul(Pr[:], lhs, wih_sb[:, k, 0:E],
                                         start=False, stop=False)
                        nc.tensor.matmul(Pz[:], lhs, wih_sb[:, k, E:2 * E],
                                         start=False, stop=False)
                        nc.tensor.matmul(Pin[:], lhs, wih_sb[:, k, 2 * E:E3],
                                         start=False, stop=(k == KCH - 1))
                return Pr, Pz, Pin, Phn

            issue_gather(0)
            if nblk > 1:
                issue_gather(1)
            cur = gi_phase(0)

            EH = E // 2          # half of the hidden dim
            H0 = slice(0, EH)
            H1 = slice(EH, E)

            for t in range(n_steps):
                Pr, Pz, Pin, Phn = cur

                # ---- gh: recurrent matmuls. r first (feeds sigmoid), then
                # hn in halves (lets d/e/tanh start on half 0 early), z last.
                for k in range(KCH if ABLATE != "chain" else 0):
                    nc.tensor.matmul(Pr[:], stT[:, k * 128:(k + 1) * 128],
                                     whh_sb[:, k, 0:E],
                                     start=False, stop=(k == KCH - 1))
                for h in ((H0, H1) if ABLATE != "chain" else ()):
                    for k in range(KCH):
                        nc.tensor.matmul(
                            Phn[:, h], stT[:, k * 128:(k + 1) * 128],
                            whh_sb[:, k, 2 * E + h.start:2 * E + h.stop],
                            start=False, stop=(k == KCH - 1 and h is H1),
                        )
                for k in range(KCH if ABLATE != "chain" else 0):
                    nc.tensor.matmul(Pz[:], stT[:, k * 128:(k + 1) * 128],
                                     whh_sb[:, k, E:2 * E],
                                     start=False, stop=(k == KCH - 1))

                # ---- prefetch: gather two blocks ahead, gi one step ahead
                if t % GB == 0 and t // GB + 2 < nblk:
                    issue_gather(t // GB + 2)
                if t + 1 < n_steps:
                    cur = gi_phase(t + 1)

                if ABLATE == "pe":
                    continue
                # ---- gates; h0 of the f/q/s' tail on DVE, h1 on GPSIMD ----
                r_sb = wp.tile([BS, E], f32, tag="r_sb", name=f"r_{t}")
                u0_sb = wp.tile([BS, E], f32, tag="u0_sb", name=f"u0_{t}")
                d_sb = wp.tile([BS, E], f32, tag="d_sb", name=f"d_{t}")
                e_sb = wp.tile([BS, E], f32, tag="e_sb", name=f"e_{t}")
                n_sb = wp.tile([BS, E], f32, tag="n_sb", name=f"n_{t}")
                f_sb = wp.tile([BS, E], f32, tag="f_sb", name=f"f_{t}")
                q_sb = wp.tile([BS, E], f32, tag="q_sb", name=f"q_{t}")
                state_new = wp.tile([BS, E], f32, tag="state", name=f"st_{t}")
                a_col = alive_sb[:, t:t + 1]

                # ACT stream: sr0, sr1, tanh0, sz0, tanh1, sz1
                nc.scalar.activation(r_sb[:, H0], Pr[:, H0], AF.Sigmoid)
                nc.scalar.activation(r_sb[:, H1], Pr[:, H1], AF.Sigmoid)
                # DVE stream: d0 e0 d1 e1 f0 q0 s0 ...
                nc.vector.tensor_tensor(d_sb[:, H0], r_sb[:, H0], Phn[:, H0],
                                        op=OP.mult)
                nc.vector.tensor_tensor(e_sb[:, H0], d_sb[:, H0], Pin[:, H0],
                                        op=OP.add)
                nc.scalar.activation(n_sb[:, H0], e_sb[:, H0], AF.Tanh)
                nc.scalar.activation(u0_sb[:, H0], Pz[:, H0], AF.Sigmoid,
                                     scale=-1.0)
                nc.vector.tensor_tensor(d_sb[:, H1], r_sb[:, H1], Phn[:, H1],
                                        op=OP.mult)
                nc.vector.tensor_tensor(e_sb[:, H1], d_sb[:, H1], Pin[:, H1],
                                        op=OP.add)
                nc.scalar.activation(n_sb[:, H1], e_sb[:, H1], AF.Tanh)
                nc.scalar.activation(u0_sb[:, H1], Pz[:, H1], AF.Sigmoid,
                                     scale=-1.0)
                # tail half 0 on DVE
                nc.vector.tensor_tensor(f_sb[:, H0], n_sb[:, H0],
                                        state[:, H0], op=OP.subtract)
                nc.vector.tensor_tensor(q_sb[:, H0], f_sb[:, H0],
                                        u0_sb[:, H0], op=OP.mult)
                nc.vector.scalar_tensor_tensor(
                    state_new[:, H0], q_sb[:, H0], a_col, state[:, H0],
                    op0=OP.mult, op1=OP.add)
                # tail half 1 (GP_TAIL picks GPSIMD vs DVE; blend on DVE:
                # TensorScalarPtr is not a Pool-engine opcode)
                eng1 = nc.gpsimd if GP_TAIL else nc.vector
                eng1.tensor_tensor(f_sb[:, H1], n_sb[:, H1],
                                   state[:, H1], op=OP.subtract)
                eng1.tensor_tensor(q_sb[:, H1], f_sb[:, H1],
                                   u0_sb[:, H1], op=OP.mult)
                nc.vector.scalar_tensor_tensor(
                    state_new[:, H1], q_sb[:, H1], a_col, state[:, H1],
                    op0=OP.mult, op1=OP.add)
                state = state_new

                # ---- transpose state for next step's gh ----
                if t + 1 < n_steps:
                    st_bf = wp.tile([BS, E], bf16, tag="st_bf", name=f"sb_{t}")
                    nc.vector.tensor_copy(st_bf[:, H0], state[:, H0])
                    eng1.tensor_copy(st_bf[:, H1], state[:, H1])
                    stT_ps = ps.tile([128, E], bf16, tag="phnT",
                                     name=f"stTp_{t}")
                    for c in range(KCH):
                        nc.tensor.transpose(
                            stT_ps[:, c * 128:(c + 1) * 128],
                            st_bf[:, c * 128:(c + 1) * 128], ident[:],
                        )
                    stT_new = wp.tile([128, E], bf16, tag="stT",
                                      name=f"stT_{t}")
                    nc.vector.tensor_copy(stT_new[:, H0], stT_ps[:, H0])
                    nc.vector.tensor_copy(stT_new[:, H1], stT_ps[:, H1])
                    stT = stT_new

            nc.sync.dma_start(out[:], state[:])


def _body_v2(nc, tc, cp, gp, wp, ps, n_steps, env):
    """v2: merged 2-bank PSUM tiles (Prz = [r|z], Pnh = [in|hn]) written in
    same-tile instruction runs; biases via two wide K=1 matmuls; f32->bf16
    casts on ACT; state transpose via XBAR DMA (sync+scalar HWDGE rings)
    instead of PE transposes + DVE copies."""
    import concourse.mybir as mybir
    dt = mybir.dt
    f32, bf16 = dt.float32, dt.bfloat16
    AF = mybir.ActivationFunctionType
    OP = mybir.AluOpType
    wih_sb, whh_sb = env["wih_sb"], env["whh_sb"]
    brz_sb, bin_sb, bhn_sb = env["brz_sb"], env["bin_sb"], env["bhn_sb"]
    bnh_sb = env["bnh_sb"]
    alive_sb, idx_sb = env["alive_sb"], env["idx_sb"]
    ones_sb = env["ones_sb"]
    emb, out = env["emb"], env["out"]
    nblk = (n_steps + GB - 1) // GB
    E2 = 2 * E
    EH = E // 2
    H0 = slice(0, EH)
    H1 = slice(EH, E)

    # ---- initial state (zeros) ----
    state = wp.tile([BS, E], f32, tag="state")
    nc.vector.memset(state[:], 0.0)
    stT = wp.tile([128, KCH, 128], bf16, tag="stT")
    nc.vector.memset(stT[:], 0.0)

    # ---- gather blocks (prefetched) ----
    emb_blocks = [None] * nblk

    def issue_gather(g):
        et = gp.tile([128, KCH, GB * BS], bf16, tag="embT", name=f"embT_{g}")
        cols = GB * BS // 16
        nc.gpsimd.dma_gather(
            et[:], emb[:], idx_sb[:, g * cols:(g + 1) * cols],
            num_idxs=GB * BS, num_idxs_reg=GB * BS, elem_size=E,
            transpose=True,
        )
        emb_blocks[g] = et

    def gi_phase(t):
        """Biases + input-side matmuls for step t into fresh PSUM tiles,
        grouped into same-tile instruction runs (avoids the per-matmul
        output-switch bubble). Separate tiles per gate keep the PSUM
        accumulation groups independent, so the r-sigmoid doesn't wait on
        the z matmuls. Returns (Pr, Pz, Pin, Phn)."""
        Pr = ps.tile([BS, E], f32, tag="pr", name=f"pr_{t}")
        Pz = ps.tile([BS, E], f32, tag="pz", name=f"pz_{t}")
        Pin = ps.tile([BS, E], f32, tag="pin", name=f"pin_{t}")
        Phn = ps.tile([BS, E], f32, tag="phn", name=f"phn_{t}")
        et = emb_blocks[t // GB]
        s = (t % GB) * BS

        nc.tensor.matmul(Pr[:], ones_sb[:], brz_sb[:, 0:E],
                         start=True, stop=False)
        if ABLATE != "chain":
            for k in range(KCH):
                nc.tensor.matmul(Pr[:], et[:, k, s:s + BS],
                                 wih_sb[:, k, 0:E], start=False, stop=False)
        nc.tensor.matmul(Pz[:], ones_sb[:], brz_sb[:, E:E2],
                         start=True, stop=False)
        if ABLATE != "chain":
            for k in range(KCH):
                nc.tensor.matmul(Pz[:], et[:, k, s:s + BS],
                                 wih_sb[:, k, E:E2], start=False, stop=False)
        nc.tensor.matmul(Pin[:], ones_sb[:], bin_sb[:],
                         start=True, stop=(ABLATE == "chain"))
        if ABLATE != "chain":
            for k in range(KCH):
                nc.tensor.matmul(Pin[:], et[:, k, s:s + BS],
                                 wih_sb[:, k, E2:E3],
                                 start=False, stop=(k == KCH - 1))
        nc.tensor.matmul(Phn[:], ones_sb[:], bhn_sb[:],
                         start=True, stop=False)
        return Pr, Pz, Pin, Phn

    issue_gather(0)
    if nblk > 1:
        issue_gather(1)
    cur = gi_phase(0)

    for t in range(n_steps):
        Pr, Pz, Pin, Phn = cur

        # ---- gh runs: r first (feeds sigmoid early), hn halves, z last ----
        if ABLATE != "chain":
            if RHALF:
                for h in (H0, H1):
                    for k in range(KCH):
                        nc.tensor.matmul(
                            Pr[:, h], stT[:, k, :],
                            whh_sb[:, k, h.start:h.stop],
                            start=False, stop=(k == KCH - 1 and h is H1))
            else:
                for k in range(KCH):
                    nc.tensor.matmul(Pr[:], stT[:, k, :],
                                     whh_sb[:, k, 0:E],
                                     start=False, stop=(k == KCH - 1))
            for h in (H0, H1):
                for k in range(KCH):
                    nc.tensor.matmul(
                        Phn[:, h], stT[:, k, :],
                        whh_sb[:, k, E2 + h.start:E2 + h.stop],
                        start=False, stop=(k == KCH - 1 and h is H1),
                    )
            for k in range(KCH):
                nc.tensor.matmul(Pz[:], stT[:, k, :],
                                 whh_sb[:, k, E:E2],
                                 start=False, stop=(k == KCH - 1))

        # ---- prefetch: gather two blocks ahead, gi one step ahead ----
        if t % GB == 0 and t // GB + 2 < nblk:
            issue_gather(t // GB + 2)
        if t + 1 < n_steps:
            cur = gi_phase(t + 1)

        if ABLATE == "pe":
            continue
        # ---- gates ----
        r_sb = wp.tile([BS, E], f32, tag="r_sb", name=f"r_{t}")
        u0_sb = wp.tile([BS, E], f32, tag="u0_sb", name=f"u0_{t}")
        d_sb = wp.tile([BS, E], f32, tag="d_sb", name=f"d_{t}")
        e_sb = wp.tile([BS, E], f32, tag="e_sb", name=f"e_{t}")
        n_sb = wp.tile([BS, E], f32, tag="n_sb", name=f"n_{t}")
        f_sb = wp.tile([BS, E], f32, tag="f_sb", name=f"f_{t}")
        q_sb = wp.tile([BS, E], f32, tag="q_sb", name=f"q_{t}")
        state_new = wp.tile([BS, E], f32, tag="state", name=f"st_{t}")
        a_col = alive_sb[:, t:t + 1]

        Pr0, Pr1 = Pr[:, H0], Pr[:, H1]
        Pz0, Pz1 = Pz[:, H0], Pz[:, H1]
        Pi0, Pi1 = Pin[:, H0], Pin[:, H1]
        Ph0, Ph1 = Phn[:, H0], Phn[:, H1]

        # ACT stream: sr0, sr1, tanh0, sz0, tanh1, sz1, cast0, cast1
        nc.scalar.activation(r_sb[:, H0], Pr0, AF.Sigmoid)
        nc.scalar.activation(r_sb[:, H1], Pr1, AF.Sigmoid)
        # DVE stream: d0 e0 d1 e1 f0 q0 s0 f1 q1 s1
        nc.vector.tensor_tensor(d_sb[:, H0], r_sb[:, H0], Ph0, op=OP.mult)
        nc.vector.tensor_tensor(e_sb[:, H0], d_sb[:, H0], Pi0, op=OP.add)
        nc.scalar.activation(n_sb[:, H0], e_sb[:, H0], AF.Tanh)
        nc.scalar.activation(u0_sb[:, H0], Pz0, AF.Sigmoid, scale=-1.0)
        nc.vector.tensor_tensor(d_sb[:, H1], r_sb[:, H1], Ph1, op=OP.mult)
        nc.vector.tensor_tensor(e_sb[:, H1], d_sb[:, H1], Pi1, op=OP.add)
        nc.scalar.activation(n_sb[:, H1], e_sb[:, H1], AF.Tanh)
        nc.scalar.activation(u0_sb[:, H1], Pz1, AF.Sigmoid, scale=-1.0)
        nc.vector.tensor_tensor(f_sb[:, H0], n_sb[:, H0], state[:, H0],
                                op=OP.subtract)
        nc.vector.tensor_tensor(q_sb[:, H0], f_sb[:, H0], u0_sb[:, H0],
                                op=OP.mult)
        nc.vector.scalar_tensor_tensor(
            state_new[:, H0], q_sb[:, H0], a_col, state[:, H0],
            op0=OP.mult, op1=OP.add)
        eng1 = nc.gpsimd if GP_TAIL else nc.vector
        eng1.tensor_tensor(f_sb[:, H1], n_sb[:, H1], state[:, H1],
                           op=OP.subtract)
        eng1.tensor_tensor(q_sb[:, H1], f_sb[:, H1], u0_sb[:, H1],
                           op=OP.mult)
        nc.vector.scalar_tensor_tensor(
            state_new[:, H1], q_sb[:, H1], a_col, state[:, H1],
            op0=OP.mult, op1=OP.add)
        state = state_new

        # ---- transpose state for next step's gh: cast on DVE, then one
        # XBAR DMA transpose on the (otherwise idle) sync HWDGE ring ----
        if t + 1 < n_steps:
            st_bf = wp.tile([BS, E], bf16, tag="st_bf", name=f"sb_{t}")
            nc.vector.tensor_copy(st_bf[:, H0], state[:, H0])
            eng1.tensor_copy(st_bf[:, H1], state[:, H1])
            stT_new = wp.tile([128, KCH, 128], bf16, tag="stT",
                              name=f"stT_{t}")
            if XBAR:
                nc.sync.dma_start(stT_new[:], st_bf[:], transpose=True)
            else:
                ident = env["ident"]
                stT_ps = ps.tile([128, E], bf16, tag="phn",
                                 name=f"stTp_{t}")
                for c in range(KCH):
                    nc.tensor.transpose(
                        stT_ps[:, c * 128:(c + 1) * 128],
                        st_bf[:, c * 128:(c + 1) * 128], ident[:],
                    )
                nc.vector.tensor_copy(stT_new[:, 0:KCH // 2, :],
                                      stT_ps[:, 0:E // 2])
                nc.vector.tensor_copy(stT_new[:, KCH // 2:KCH, :],
                                      stT_ps[:, E // 2:E])
            stT = stT_new

    nc.sync.dma_start(out[:], state[:])


def _get_nc(n_steps, repeat=1):
    key = (n_steps, repeat, IMPL, WIDE_MM, XBAR, RHALF)
    if key not in _nc_cache:
        _nc_cache[key] = _build(n_steps, repeat)
    return _nc_cache[key]


def _prep_inputs(utterance, emb_table, W_ih, W_hh, b_ih, b_hh, term_id):
    """Host-side sharding/layout prep. Returns per-core in_maps."""
    utterance = np.asarray(utterance, dtype=np.int32)
    emb_table = np.asarray(emb_table, dtype=np.float32)
    W_ih = np.asarray(W_ih, dtype=np.float32)
    W_hh = np.asarray(W_hh, dtype=np.float32)
    b_ih = np.asarray(b_ih, dtype=np.float32)
    b_hh = np.asarray(b_hh, dtype=np.float32)
    term = int(np.asarray(term_id))

    bf = ml_dtypes.bfloat16
    emb_bf = np.ascontiguousarray(emb_table.astype(bf))

    def wprep(W):  # [3E, E] -> [128, KCH, 3E] with w[p,k,n] = W[n, k*128+p]
        Wt = W.T.reshape(KCH, 128, E3).transpose(1, 0, 2)
        return np.ascontiguousarray(Wt.astype(bf))

    wih_h = wprep(W_ih)
    whh_h = wprep(W_hh)
    brz_h = np.ascontiguousarray(
        (b_ih[:2 * E] + b_hh[:2 * E]).reshape(1, 2 * E).astype(bf))
    bin_h = np.ascontiguousarray(b_ih[2 * E:].reshape(1, E).astype(bf))
    bhn_h = np.ascontiguousarray(b_hh[2 * E:].reshape(1, E).astype(bf))
    bnh_h = np.ascontiguousarray(
        np.concatenate([b_ih[2 * E:], b_hh[2 * E:]]).reshape(1, 2 * E)
        .astype(bf))

    in_maps = []
    for c in range(NCORES):
        U = utterance[:, c * BS:(c + 1) * BS]          # [M, BS], (t, b)
        flat = U.reshape(-1).astype(np.int16)           # i = t*BS + b
        idx_h = np.ascontiguousarray(np.tile(flat.reshape(-1, 16).T, (8, 1)))  # [128, M*BS/16]
        hit = (U == term)
        csum = np.cumsum(hit, axis=0)
        aliveT = np.ones((M, BS), dtype=np.float32)
        aliveT[1:] = (csum[:-1] == 0)
        alive_h = np.ascontiguousarray(aliveT.T)        # [BS, M]
        in_maps.append({
            "emb": emb_bf, "idx": idx_h, "alive": alive_h,
            "wih": wih_h, "whh": whh_h,
            "brz": brz_h, "bin": bin_h, "bhn": bhn_h, "bnh": bnh_h,
        })
    return in_maps


def kernel(utterance, emb_table, W_ih, W_hh, b_ih, b_hh, term_id,
           n_steps=M):
    from concourse.bass_utils import run_bass_kernel_spmd

    nc = _get_nc(n_steps)
    in_maps = _prep_inputs(utterance, emb_table, W_ih, W_hh, b_ih, b_hh,
                           term_id)
    res = run_bass_kernel_spmd(nc, in_maps, core_ids=list(range(NCORES)),
                               trace=TRACE)
    LAST_RESULT["exec_time_ns"] = res.exec_time_ns
    LAST_RESULT["trace"] = res.instructions_and_trace
    return np.concatenate([r["out"] for r in res.results], axis=0)

